# revision 5
# baseline (speedup 1.0000x reference)
"""Causal single-head attention (S=8192, dk=64) on 8 TRN2 NeuronCores.

Sharding: zigzag sequence-parallel over query rows. The 8192 rows form 16
blocks of 512; core b owns row-blocks {b, 15-b} so every core does exactly
17 block-sized (512 rows x 512 keys) units of causal work -> perfect load
balance, no collectives.

SPMD constraint (all cores share one instruction graph) is satisfied by
host-side packing: the host packs, per core, 17 "slots" of
(qT, kT, v_aug) operand tiles; slots 0 and 1 are the two diagonal
(triangular-masked) blocks for every core, the remaining 15 are full
blocks. The device graph is identical across cores; only data differs.
Slots are processed as 9 pairs (slot i, slot 9+i) mapped to the two
PE-array row-halves so the K=64 QK^T matmuls run two-at-a-time.

Device pipeline per pair (Tile framework handles sync):
  QK^T: per key-subtile: two concurrent matmuls [K=64, M=128, N<=512]
        (tile_position (0,0)/(64,0)) -> sT [128,<=1024] f32 in PSUM.
        Diagonal slots skip fully-masked query rows (N=512-roff).
  exp:  exp(s/64) -> bf16 SBUF; kt tiles alternate between the Vector
        engine (custom (cubic)^2 DVE op) and Scalar ACT (exact, fused
        scale) so two exps are in flight and both engines share the
        per-element softmax work evenly.
  mask: slots 0/1 only: gpsimd affine_select zeroes key>row entries.
  AV:   per slot: 4 matmuls lhsT=v_aug[128 keys, 65] rhs=exp tile,
        accumulated in a per-slot PSUM tile [65, 512] (one bank, 2 bufs
        per slot-lane) so a pair's AV is never serialized behind the
        previous pair's PSUM->SBUF copy; row 64 of v_aug is ones so row
        64 is the softmax denominator.
  out:  per slot: one PSUM->SBUF copy (slot a on Vector, slot b on
        Scalar) casting f32->bf16, then one DMA per pair (bf16 halves
        the output DMA bytes).

Input DMAs are fused per pair (q^T|k^T|v_aug strips in one contiguous
DRAM row); the first pair's q+k leading columns get a dedicated small
DMA so the first matmul's dependency lands as early as possible.

Host combines: per row-chunk, sum slot partials (f32), divide by the
denominator row.
"""

import numpy as np
import ml_dtypes

S = 8192
DK = 64
BLK = 512  # row/key block
NB = S // BLK  # 16
N_CORES = 8
NSLOT = 17  # (b+1) + (16-b) block units per core
G0 = 9  # slots 0..8 -> PE rows 0:64, slots 9..16 -> PE rows 64:128
NPAIR = 9
KSUB = 128  # key subtile (psum partition dim)
NKT = BLK // KSUB  # 4
VW = NKT * 65  # 260
QKW = 2 * BLK  # 1024 cols of q|k per pair
INW = QKW + 2 * VW  # 1544 cols per fused input row

# diagonal pairs (0,1) have longer exp->mask->AV chains: keep them off the
# cold start and off the drain tail
PAIR_ORDER = [2, 3, 0, 1, 4, 5, 6, 7, 8]

_BF16 = ml_dtypes.bfloat16
_CACHE = {}

# cubic-in-t fit of exp(t/128) (chebyshev nodes, |t|<=56); the DVE op
# squares it to get exp(t/64). Max rel err ~5.5e-4 for |t|<=56.
_EXPC = (8.02364796e-08, 3.10070749e-05, 7.81220049e-03, 9.99807965e-01)


def _register_exp_dve_op():
    """Register a custom DVE op: out = (((x*c3 + c2)*x + c1)*x + c0)^2.

    One DVE instruction evaluates exp(x/64) to ~5e-4 rel err, letting the
    Vector engine share softmax-exp work with the Scalar engine (the
    per-element-throughput bottleneck of this kernel).
    """
    import numpy as np
    from concourse import dve_ops
    from concourse.dve_spec import (
        Spec, Src0, C0, C1, C2, C3, _spill_c3_to_src1, lower, _has_src1, sq,
    )
    from concourse.dve_uop import DveOpSpec

    name = "EXP_SQ_ANT"
    if name in dve_ops._SUB_OPCODE_FOR_NAME:
        return next(o for o in dve_ops.OPS if o.name == name)

    body = _spill_c3_to_src1(
        sq(((Src0 * C0 + C1) * Src0 + C2) * Src0 + C3))

    def ref(in0, in1, s0, s1, imm2):
        x = in0.astype(np.float32)
        p = ((x * s0 + s1) * x + imm2) * x + in1
        return (p * p).astype(np.float32)

    spec = Spec(body=body, reference=ref)
    row = dve_ops._CUSTOM_DVE_ROW_BASE + len(dve_ops.OPS)
    assert row < 0x20
    shas = {}
    for ver in ("v3",):
        s = DveOpSpec(name=name, opcode=row, uops=lower(spec, ver=ver),
                      rd1_en=_has_src1(spec))
        shas[ver] = s.sha(ver)
    op = dve_ops.DveOp(name, spec, subdim=False, uops_sha=shas)
    dve_ops.OPS.append(op)
    dve_ops._SUB_OPCODE_FOR_NAME[name] = row
    dve_ops.CUSTOM_DVE_SPECS[name] = spec
    return op


def _core_slots(b):
    """Slot table for core b: list of (rowblock, keyblock, is_diag)."""
    A, B = b, 15 - b
    slots = [(A, A, True), (B, B, True)]
    slots += [(A, c, False) for c in range(A)]
    slots += [(B, c, False) for c in range(B)]
    assert len(slots) == NSLOT
    return slots


def _build_graph():
    import concourse.mybir as mybir
    import concourse.tile as tile
    from concourse import bacc

    f32 = mybir.dt.float32
    bf16 = mybir.dt.bfloat16

    exp_op = _register_exp_dve_op()
    d3, d2, d1, d0 = _EXPC

    nc = bacc.Bacc("TRN2", target_bir_lowering=False)
    # fused per-pair input rows, laid out in PAIR_ORDER position order:
    # cols [0:512) q^T  [512:1024) k^T  [1024:1544) v_aug strips
    inp = nc.declare_dram_parameter("inp", [NPAIR, 128, INW], bf16,
                                    isOutput=False)
    op = nc.declare_dram_parameter("op", [NPAIR, 65, 2 * BLK], bf16,
                                   isOutput=True)

    with tile.TileContext(nc) as tc:
        with (
            tc.tile_pool(name="data", bufs=1) as data,
            tc.tile_pool(name="stp", bufs=3, space="PSUM") as stp,
            tc.tile_pool(name="avp", bufs=2, space="PSUM") as avp,
            tc.tile_pool(name="sxp", bufs=10) as sxp,
            tc.tile_pool(name="outp", bufs=2) as outp,
        ):
            # fused input tiles; the first pair's qk leading columns
            # (q full + k kt0 strip) get their own DMA so the first
            # matmul's dependency is a single small transfer
            incol = {}
            for pos, i in enumerate(PAIR_ORDER):
                t = data.tile([128, INW], bf16, tag=f"in{i}", name=f"in{i}")
                if pos == 0:
                    nc.sync.dma_start(out=t[:, 0:BLK + KSUB],
                                      in_=inp[pos][:, 0:BLK + KSUB])
                    nc.sync.dma_start(out=t[:, BLK + KSUB:QKW],
                                      in_=inp[pos][:, BLK + KSUB:QKW])
                    nc.sync.dma_start(out=t[:, QKW:INW],
                                      in_=inp[pos][:, QKW:INW])
                else:
                    nc.sync.dma_start(out=t, in_=inp[pos])
                incol[i] = t
            d0col = data.tile([128, 1], f32, tag="d0col", name="d0col")
            nc.vector.memset(d0col, d0)

            def emit_av_out(pos, i, slots, sxs, last=False):
                """AV matmuls + per-slot copies + output DMA for a pair.

                The two slots' AV matmuls are interleaved per key-subtile
                so each weight load overlaps the other slot's streaming
                matmul (hides the accumulation-group-start LDW stall).
                """
                ot = outp.tile([65, 2 * BLK], bf16, tag="ot", name=f"ot{i}")
                avs = {}
                for s in slots:
                    avs[s] = avp.tile([65, BLK], f32, tag="av",
                                      name=f"av{i}s{s}")
                for kt in range(NKT):
                    for s in slots:
                        roff = KSUB * kt if s < 2 else 0
                        voff = QKW + (0 if s == i else VW)
                        nc.tensor.matmul(
                            avs[s][:, roff:BLK],
                            incol[i][:, voff + kt * 65:voff + kt * 65 + 65],
                            sxs[s][kt][:, roff:BLK],
                            start=(kt == 0),
                            stop=(kt == NKT - 1),
                        )
                for s in slots:
                    off = 0 if s == i else BLK
                    if last:
                        # final pair: split the copy across both engines
                        # to shorten the drain tail
                        h = BLK // 2
                        nc.vector.tensor_copy(ot[:, off:off + h],
                                              avs[s][:, 0:h])
                        nc.scalar.copy(ot[:, off + h:off + BLK],
                                       avs[s][:, h:BLK])
                    elif s == i:
                        # slot a -> Vector, slot b -> Scalar: balances the
                        # PSUM->SBUF copy work across the two engines
                        nc.vector.tensor_copy(ot[:, off:off + BLK], avs[s])
                    else:
                        nc.scalar.copy(ot[:, off:off + BLK], avs[s])
                w = 2 * BLK if len(slots) == 2 else BLK
                nc.sync.dma_start(out=op[pos][:, 0:w], in_=ot[:, 0:w])

            # software-pipelined emission: pair p's AV stage is emitted
            # after pair p+1's QK+exp stage, so in the PE queue the two
            # row-half QK matmuls of each subtile stay adjacent (they run
            # concurrently in the array) and AVs fill dependency stalls
            prev = None
            for pos, i in enumerate(PAIR_ORDER):
                slots = [i] + ([9 + i] if 9 + i < NSLOT else [])
                sxs = {s: [] for s in slots}
                for kt in range(NKT):
                    # one PSUM tile holds this subtile's scores for BOTH
                    # slots of the pair (one bank each) -> single pool
                    # allocation, so the two row-half matmuls stay adjacent
                    # in the PE queue and run concurrently in the array
                    st = stp.tile([128, 2 * BLK], f32, tag="st",
                                  name=f"st{i}k{kt}")
                    # diagonal slots (0/1, always in the g0 half): rows
                    # < 128*kt of subtile kt are fully masked - skip them
                    # in QK, exp and AV
                    roff = KSUB * kt if i < 2 else 0
                    for s in slots:
                        p0 = 0 if s < G0 else 64
                        off = 0 if s == i else BLK
                        ro = roff if s == i else 0
                        nc.tensor.matmul(
                            st[:, off + ro:off + BLK],
                            incol[i][p0:p0 + 64,
                                     BLK + kt * KSUB:BLK + (kt + 1) * KSUB],
                            incol[i][p0:p0 + 64, ro:BLK],
                            start=True,
                            stop=True,
                            tile_position=(p0, 0),
                        )
                    sx = sxp.tile([128, 2 * BLK], bf16, tag="sx",
                                  name=f"sx{i}k{kt}")
                    if len(slots) == 2 and roff == 0:
                        spans = [(0, 2 * BLK)]
                    elif len(slots) == 2:
                        spans = [(roff, BLK), (BLK, 2 * BLK)]
                    else:
                        spans = [(roff, BLK)]
                    # kt0/kt2 -> Scalar (exact exp; ACT is the faster and
                    # less-loaded engine, and QK kt2/kt3 depend on these
                    # buffers freeing), kt1/kt3 -> Vector (cubic^2 custom
                    # op): two exps in flight per pair and an even
                    # per-pair split of the elementwise work
                    for lo, hi in spans:
                        if kt % 2 == 1:
                            nc.vector._custom_dve(
                                exp_op, out=sx[:, lo:hi],
                                in0=st[:, lo:hi], in1=d0col,
                                s0=d3, s1=d2, imm2=d1,
                            )
                        else:
                            nc.scalar.activation(
                                sx[:, lo:hi], st[:, lo:hi],
                                mybir.ActivationFunctionType.Exp,
                                scale=1.0 / DK,
                            )
                    if i < 2:  # diagonal slot: zero keys > row
                        nc.gpsimd.affine_select(
                            out=sx[:, roff:BLK],
                            in_=sx[:, roff:BLK],
                            pattern=[[1, BLK - roff]],
                            compare_op=mybir.AluOpType.is_ge,
                            fill=0.0,
                            base=0,
                            channel_multiplier=-1,
                        )
                    for s in slots:
                        off = 0 if s == i else BLK
                        sxs[s].append(sx[:, off:off + BLK])
                    # emit the previous pair's AV stage mid-way through
                    # this pair's QK stage: kt0/kt1 fill the PE while the
                    # previous exps drain, and kt2/kt3 (which depend on
                    # this pair's own exps) queue after the AVs
                    if kt == 1 and prev is not None:
                        emit_av_out(*prev)
                prev = (pos, i, slots, sxs)
            emit_av_out(*prev, last=True)

    nc.finalize()
    return nc


def _pack_core(q_bf, k_bf, v_bf, b):
    """Build the packed fused operand array for core b."""
    inp = np.zeros((NPAIR, 128, INW), dtype=_BF16)
    slots = _core_slots(b)
    pos_of = {i: pos for pos, i in enumerate(PAIR_ORDER)}
    for s, (rb, cb, _diag) in enumerate(slots):
        i = s if s < G0 else s - G0
        pos = pos_of[i]
        p0 = 0 if s < G0 else 64
        voff = QKW + (0 if s < G0 else VW)
        inp[pos, p0:p0 + 64, 0:BLK] = q_bf[rb * BLK:(rb + 1) * BLK].T
        inp[pos, p0:p0 + 64, BLK:QKW] = k_bf[cb * BLK:(cb + 1) * BLK].T
        for kt in range(NKT):
            c0 = voff + kt * 65
            inp[pos, :, c0:c0 + 64] = (
                v_bf[cb * BLK + kt * KSUB: cb * BLK + (kt + 1) * KSUB])
            inp[pos, :, c0 + 64] = np.asarray(1.0, dtype=_BF16)
    return {"inp": inp}


def _slot_partial(op_arr, s):
    """Extract slot s's [65, 512] partial from the per-pair output array."""
    i = s if s < G0 else s - G0
    pos = PAIR_ORDER.index(i)
    off = 0 if s < G0 else BLK
    return op_arr[pos, :, off:off + BLK]


def _combine(partials):
    """partials: list of 8 arrays [9, 65, 1024] -> full [8192, 64] f32."""
    out = np.empty((S, DK), dtype=np.float32)
    for b in range(N_CORES):
        slots = _core_slots(b)
        for rb in (b, 15 - b):
            idx = [s for s, (r, _c, _d) in enumerate(slots) if r == rb]
            tot = np.zeros((65, BLK), dtype=np.float32)
            for s in idx:
                tot += _slot_partial(partials[b], s).astype(np.float32)
            out[rb * BLK:(rb + 1) * BLK] = (tot[:DK] / tot[DK]).T
    return out


def kernel(q, k, v):
    from concourse.bass_utils import run_bass_kernel_spmd

    q = np.asarray(q, dtype=np.float32)
    k = np.asarray(k, dtype=np.float32)
    v = np.asarray(v, dtype=np.float32)

    if "nc" not in _CACHE:
        _CACHE["nc"] = _build_graph()
    nc = _CACHE["nc"]

    q_bf = q.astype(_BF16)
    k_bf = k.astype(_BF16)
    v_bf = v.astype(_BF16)
    in_maps = [_pack_core(q_bf, k_bf, v_bf, b) for b in range(N_CORES)]

    res = run_bass_kernel_spmd(nc, in_maps, core_ids=list(range(N_CORES)))
    partials = [np.asarray(res.results[b]["op"]) for b in range(N_CORES)]
    return _combine(partials)


# revision 9
# speedup vs baseline: 1.0155x; 1.0155x over previous
"""Causal single-head attention (S=8192, dk=64) on 8 TRN2 NeuronCores.

Sharding: zigzag sequence-parallel over query rows. The 8192 rows form 16
blocks of 512; core b owns row-blocks {b, 15-b} so every core does exactly
17 block-sized (512 rows x 512 keys) units of causal work -> perfect load
balance, no collectives.

SPMD constraint (all cores share one instruction graph) is satisfied by
host-side packing: the host packs, per core, 17 "slots" of
(qT, kT, v_aug) operand tiles; slots 0 and 1 are the two diagonal
(triangular-masked) blocks for every core, the remaining 15 are full
blocks. The device graph is identical across cores; only data differs.
Slots are processed as 9 pairs (slot i, slot 9+i) mapped to the two
PE-array row-halves so the K=64 QK^T matmuls run two-at-a-time.

Device pipeline per pair (Tile framework handles sync):
  QK^T: per key-subtile: two concurrent matmuls [K=64, M=128, N<=512]
        (tile_position (0,0)/(64,0)) -> sT [128,<=1024] f32 in PSUM.
        Diagonal slots skip fully-masked query rows (N=512-roff).
  exp:  exp(s/64) -> bf16 SBUF; kt tiles alternate between the Vector
        engine (custom (cubic)^2 DVE op) and Scalar ACT (exact, fused
        scale) so two exps are in flight and both engines share the
        per-element softmax work evenly.
  mask: slots 0/1 only: gpsimd affine_select zeroes key>row entries.
  AV:   per slot: 4 matmuls lhsT=v_aug[128 keys, 65] rhs=exp tile,
        accumulated in a per-slot PSUM tile [65, 512] (one bank, 2 bufs
        per slot-lane) so a pair's AV is never serialized behind the
        previous pair's PSUM->SBUF copy; row 64 of v_aug is ones so row
        64 is the softmax denominator.
  out:  per slot: one PSUM->SBUF copy (slot a on Vector, slot b on
        Scalar) casting f32->bf16, then one DMA per pair (bf16 halves
        the output DMA bytes).

Input DMAs are fused per pair (q^T|k^T|v_aug strips in one contiguous
DRAM row); the first pair's q+k leading columns get a dedicated small
DMA so the first matmul's dependency lands as early as possible.

Host combines: per row-chunk, sum slot partials (f32), divide by the
denominator row.
"""

import numpy as np
import ml_dtypes

S = 8192
DK = 64
BLK = 512  # row/key block
NB = S // BLK  # 16
N_CORES = 8
NSLOT = 17  # (b+1) + (16-b) block units per core
G0 = 9  # slots 0..8 -> PE rows 0:64, slots 9..16 -> PE rows 64:128
NPAIR = 9
KSUB = 128  # key subtile (psum partition dim)
NKT = BLK // KSUB  # 4
VW = NKT * 65  # 260
QKW = 2 * BLK  # 1024 cols of q|k per pair
INW = QKW + 2 * VW  # 1544 cols per fused input row

# diagonal pairs (0,1) have longer exp->mask->AV chains: keep them off the
# cold start and off the drain tail
PAIR_ORDER = [2, 3, 0, 1, 4, 5, 6, 7, 8]

_BF16 = ml_dtypes.bfloat16
_CACHE = {}

# cubic-in-t fit of exp(t/128) (chebyshev nodes, |t|<=56); the DVE op
# squares it to get exp(t/64). Max rel err ~5.5e-4 for |t|<=56.
_EXPC = (8.02364796e-08, 3.10070749e-05, 7.81220049e-03, 9.99807965e-01)


def _register_exp_dve_op():
    """Register a custom DVE op: out = (((x*c3 + c2)*x + c1)*x + c0)^2.

    One DVE instruction evaluates exp(x/64) to ~5e-4 rel err, letting the
    Vector engine share softmax-exp work with the Scalar engine (the
    per-element-throughput bottleneck of this kernel).
    """
    import numpy as np
    from concourse import dve_ops
    from concourse.dve_spec import (
        Spec, Src0, C0, C1, C2, C3, _spill_c3_to_src1, lower, _has_src1, sq,
    )
    from concourse.dve_uop import DveOpSpec

    name = "EXP_SQ_ANT"
    if name in dve_ops._SUB_OPCODE_FOR_NAME:
        return next(o for o in dve_ops.OPS if o.name == name)

    body = _spill_c3_to_src1(
        sq(((Src0 * C0 + C1) * Src0 + C2) * Src0 + C3))

    def ref(in0, in1, s0, s1, imm2):
        x = in0.astype(np.float32)
        p = ((x * s0 + s1) * x + imm2) * x + in1
        return (p * p).astype(np.float32)

    spec = Spec(body=body, reference=ref)
    row = dve_ops._CUSTOM_DVE_ROW_BASE + len(dve_ops.OPS)
    assert row < 0x20
    shas = {}
    for ver in ("v3",):
        s = DveOpSpec(name=name, opcode=row, uops=lower(spec, ver=ver),
                      rd1_en=_has_src1(spec))
        shas[ver] = s.sha(ver)
    op = dve_ops.DveOp(name, spec, subdim=False, uops_sha=shas)
    dve_ops.OPS.append(op)
    dve_ops._SUB_OPCODE_FOR_NAME[name] = row
    dve_ops.CUSTOM_DVE_SPECS[name] = spec
    return op


def _core_slots(b):
    """Slot table for core b: list of (rowblock, keyblock, is_diag)."""
    A, B = b, 15 - b
    slots = [(A, A, True), (B, B, True)]
    slots += [(A, c, False) for c in range(A)]
    slots += [(B, c, False) for c in range(B)]
    assert len(slots) == NSLOT
    return slots


def _build_graph():
    import concourse.mybir as mybir
    import concourse.tile as tile
    from concourse import bacc

    f32 = mybir.dt.float32
    bf16 = mybir.dt.bfloat16

    exp_op = _register_exp_dve_op()
    d3, d2, d1, d0 = _EXPC

    nc = bacc.Bacc("TRN2", target_bir_lowering=False)
    # fused per-pair input rows, laid out in PAIR_ORDER position order:
    # cols [0:512) q^T  [512:1024) k^T  [1024:1544) v_aug strips
    inp = nc.declare_dram_parameter("inp", [NPAIR, 128, INW], bf16,
                                    isOutput=False)
    op = nc.declare_dram_parameter("op", [NPAIR, 65, 2 * BLK], bf16,
                                   isOutput=True)

    with tile.TileContext(nc) as tc:
        with (
            tc.tile_pool(name="data", bufs=1) as data,
            tc.tile_pool(name="stp", bufs=3, space="PSUM") as stp,
            tc.tile_pool(name="avp", bufs=2, space="PSUM") as avp,
            tc.tile_pool(name="sxp", bufs=10) as sxp,
            tc.tile_pool(name="outp", bufs=2) as outp,
        ):
            # fused input tiles; the first pair's qk leading columns
            # (q full + k kt0 strip) get their own DMA so the first
            # matmul's dependency is a single small transfer
            incol = {}
            for pos, i in enumerate(PAIR_ORDER):
                t = data.tile([128, INW], bf16, tag=f"in{i}", name=f"in{i}")
                if pos == 0:
                    # the very first QK matmul (slot a, PE rows 0:64) only
                    # needs partitions 0:64 of the q + k-kt0 columns: give
                    # that 80KB strip its own DMA so compute starts ASAP
                    nc.sync.dma_start(out=t[0:64, 0:BLK + KSUB],
                                      in_=inp[pos][0:64, 0:BLK + KSUB])
                    nc.sync.dma_start(out=t[64:128, 0:BLK + KSUB],
                                      in_=inp[pos][64:128, 0:BLK + KSUB])
                    nc.sync.dma_start(out=t[:, BLK + KSUB:QKW],
                                      in_=inp[pos][:, BLK + KSUB:QKW])
                    nc.sync.dma_start(out=t[:, QKW:INW],
                                      in_=inp[pos][:, QKW:INW])
                else:
                    nc.sync.dma_start(out=t, in_=inp[pos])
                incol[i] = t
            d0col = data.tile([128, 1], f32, tag="d0col", name="d0col")
            nc.vector.memset(d0col, d0)

            def emit_av_out(pos, i, slots, sxs, last=False):
                """AV matmuls + per-slot copies + output DMA for a pair."""
                ot = outp.tile([65, 2 * BLK], bf16, tag="ot", name=f"ot{i}")
                for s in slots:
                    av = avp.tile([65, BLK], f32, tag="av",
                                  name=f"av{i}s{s}")
                    off = 0 if s == i else BLK
                    voff = QKW + (0 if s == i else VW)
                    for kt in range(NKT):
                        roff = KSUB * kt if s < 2 else 0
                        nc.tensor.matmul(
                            av[:, roff:BLK],
                            incol[i][:, voff + kt * 65:voff + kt * 65 + 65],
                            sxs[s][kt][:, roff:BLK],
                            start=(kt == 0),
                            stop=(kt == NKT - 1),
                        )
                    if last:
                        # final pair: split the copy across both engines
                        # to shorten the drain tail
                        h = BLK // 2
                        nc.vector.tensor_copy(ot[:, off:off + h],
                                              av[:, 0:h])
                        nc.scalar.copy(ot[:, off + h:off + BLK],
                                       av[:, h:BLK])
                    elif s == i:
                        # slot a -> Vector, slot b -> Scalar: balances the
                        # PSUM->SBUF copy work across the two engines
                        nc.vector.tensor_copy(ot[:, off:off + BLK], av)
                    else:
                        nc.scalar.copy(ot[:, off:off + BLK], av)
                w = 2 * BLK if len(slots) == 2 else BLK
                nc.sync.dma_start(out=op[pos][:, 0:w], in_=ot[:, 0:w])

            # software-pipelined emission: pair p's AV stage is emitted
            # after pair p+1's QK+exp stage, so in the PE queue the two
            # row-half QK matmuls of each subtile stay adjacent (they run
            # concurrently in the array) and AVs fill dependency stalls
            prev = None
            for pos, i in enumerate(PAIR_ORDER):
                slots = [i] + ([9 + i] if 9 + i < NSLOT else [])
                sxs = {s: [] for s in slots}
                for kt in range(NKT):
                    # one PSUM tile holds this subtile's scores for BOTH
                    # slots of the pair (one bank each) -> single pool
                    # allocation, so the two row-half matmuls stay adjacent
                    # in the PE queue and run concurrently in the array
                    st = stp.tile([128, 2 * BLK], f32, tag="st",
                                  name=f"st{i}k{kt}")
                    # diagonal slots (0/1, always in the g0 half): rows
                    # < 128*kt of subtile kt are fully masked - skip them
                    # in QK, exp and AV
                    roff = KSUB * kt if i < 2 else 0
                    for s in slots:
                        p0 = 0 if s < G0 else 64
                        off = 0 if s == i else BLK
                        ro = roff if s == i else 0
                        nc.tensor.matmul(
                            st[:, off + ro:off + BLK],
                            incol[i][p0:p0 + 64,
                                     BLK + kt * KSUB:BLK + (kt + 1) * KSUB],
                            incol[i][p0:p0 + 64, ro:BLK],
                            start=True,
                            stop=True,
                            tile_position=(p0, 0),
                        )
                    sx = sxp.tile([128, 2 * BLK], bf16, tag="sx",
                                  name=f"sx{i}k{kt}")
                    # diag pairs: one merged span (roff, 2*BLK) — rows
                    # [0, roff) of the diag slot hold only stale unread
                    # PSUM, so a single instruction saves the per-op
                    # overhead of a split
                    if len(slots) == 2:
                        spans = [(roff, 2 * BLK)]
                    else:
                        spans = [(roff, BLK)]
                    # kt0/kt2 -> Vector (cubic^2 custom op), kt1/kt3 ->
                    # Scalar (exact): two exps in flight per pair and an
                    # even per-pair split of the elementwise work
                    for lo, hi in spans:
                        if kt % 2 == 0:
                            nc.vector._custom_dve(
                                exp_op, out=sx[:, lo:hi],
                                in0=st[:, lo:hi], in1=d0col,
                                s0=d3, s1=d2, imm2=d1,
                            )
                        else:
                            nc.scalar.activation(
                                sx[:, lo:hi], st[:, lo:hi],
                                mybir.ActivationFunctionType.Exp,
                                scale=1.0 / DK,
                            )
                    if i < 2:  # diagonal slot: zero keys > row
                        nc.gpsimd.affine_select(
                            out=sx[:, roff:BLK],
                            in_=sx[:, roff:BLK],
                            pattern=[[1, BLK - roff]],
                            compare_op=mybir.AluOpType.is_ge,
                            fill=0.0,
                            base=0,
                            channel_multiplier=-1,
                        )
                    for s in slots:
                        off = 0 if s == i else BLK
                        sxs[s].append(sx[:, off:off + BLK])
                if prev is not None:
                    emit_av_out(*prev)
                prev = (pos, i, slots, sxs)
            emit_av_out(*prev, last=True)

    nc.finalize()
    return nc


def _pack_core(q_bf, k_bf, v_bf, b):
    """Build the packed fused operand array for core b."""
    inp = np.zeros((NPAIR, 128, INW), dtype=_BF16)
    slots = _core_slots(b)
    pos_of = {i: pos for pos, i in enumerate(PAIR_ORDER)}
    for s, (rb, cb, _diag) in enumerate(slots):
        i = s if s < G0 else s - G0
        pos = pos_of[i]
        p0 = 0 if s < G0 else 64
        voff = QKW + (0 if s < G0 else VW)
        inp[pos, p0:p0 + 64, 0:BLK] = q_bf[rb * BLK:(rb + 1) * BLK].T
        inp[pos, p0:p0 + 64, BLK:QKW] = k_bf[cb * BLK:(cb + 1) * BLK].T
        for kt in range(NKT):
            c0 = voff + kt * 65
            inp[pos, :, c0:c0 + 64] = (
                v_bf[cb * BLK + kt * KSUB: cb * BLK + (kt + 1) * KSUB])
            inp[pos, :, c0 + 64] = np.asarray(1.0, dtype=_BF16)
    return {"inp": inp}


def _slot_partial(op_arr, s):
    """Extract slot s's [65, 512] partial from the per-pair output array."""
    i = s if s < G0 else s - G0
    pos = PAIR_ORDER.index(i)
    off = 0 if s < G0 else BLK
    return op_arr[pos, :, off:off + BLK]


def _combine(partials):
    """partials: list of 8 arrays [9, 65, 1024] -> full [8192, 64] f32."""
    out = np.empty((S, DK), dtype=np.float32)
    for b in range(N_CORES):
        slots = _core_slots(b)
        for rb in (b, 15 - b):
            idx = [s for s, (r, _c, _d) in enumerate(slots) if r == rb]
            tot = np.zeros((65, BLK), dtype=np.float32)
            for s in idx:
                tot += _slot_partial(partials[b], s).astype(np.float32)
            out[rb * BLK:(rb + 1) * BLK] = (tot[:DK] / tot[DK]).T
    return out


def kernel(q, k, v):
    from concourse.bass_utils import run_bass_kernel_spmd

    q = np.asarray(q, dtype=np.float32)
    k = np.asarray(k, dtype=np.float32)
    v = np.asarray(v, dtype=np.float32)

    if "nc" not in _CACHE:
        _CACHE["nc"] = _build_graph()
    nc = _CACHE["nc"]

    q_bf = q.astype(_BF16)
    k_bf = k.astype(_BF16)
    v_bf = v.astype(_BF16)
    in_maps = [_pack_core(q_bf, k_bf, v_bf, b) for b in range(N_CORES)]

    res = run_bass_kernel_spmd(nc, in_maps, core_ids=list(range(N_CORES)))
    partials = [np.asarray(res.results[b]["op"]) for b in range(N_CORES)]
    return _combine(partials)


# revision 12
# speedup vs baseline: 1.0554x; 1.0392x over previous
"""Causal single-head attention (S=8192, dk=64) on 8 TRN2 NeuronCores.

Sharding: zigzag sequence-parallel over query rows. The 8192 rows form 16
blocks of 512; core b owns row-blocks {b, 15-b} so every core does exactly
17 block-sized (512 rows x 512 keys) units of causal work -> perfect load
balance, no collectives.

SPMD constraint (all cores share one instruction graph) is satisfied by
host-side packing: the host packs, per core, 17 "slots" of
(qT, kT, v_aug) operand tiles; slots 0 and 1 are the two diagonal
(triangular-masked) blocks for every core, the remaining 15 are full
blocks. The device graph is identical across cores; only data differs.
Slots are processed as 9 pairs (slot i, slot 9+i) mapped to the two
PE-array row-halves so the K=64 QK^T matmuls run two-at-a-time.

Device pipeline per pair (Tile framework handles sync):
  QK^T: per key-subtile: two concurrent matmuls [K=64, M=128, N<=512]
        (tile_position (0,0)/(64,0)) -> sT [128,<=1024] f32 in PSUM.
        Diagonal slots skip fully-masked query rows (N=512-roff).
  exp:  exp(s/64) -> bf16 SBUF; kt tiles alternate between the Vector
        engine (custom (cubic)^2 DVE op) and Scalar ACT (exact, fused
        scale) so two exps are in flight and both engines share the
        per-element softmax work evenly.
  mask: slots 0/1 only: gpsimd affine_select zeroes key>row entries.
  AV:   per slot: 4 matmuls lhsT=v_aug[128 keys, 65] rhs=exp tile,
        accumulated in a per-slot PSUM tile [65, 512] (one bank, 2 bufs
        per slot-lane) so a pair's AV is never serialized behind the
        previous pair's PSUM->SBUF copy; row 64 of v_aug is ones so row
        64 is the softmax denominator.
  out:  per slot: one PSUM->SBUF copy (slot a on Vector, slot b on
        Scalar) casting f32->bf16, then one DMA per pair (bf16 halves
        the output DMA bytes).

Input DMAs are fused per pair (q^T|k^T|v_aug strips in one contiguous
DRAM row); the first pair's q+k leading columns get a dedicated small
DMA so the first matmul's dependency lands as early as possible.

Host combines: per row-chunk, sum slot partials (f32), divide by the
denominator row.
"""

import numpy as np
import ml_dtypes

S = 8192
DK = 64
BLK = 512  # row/key block
NB = S // BLK  # 16
N_CORES = 8
NSLOT = 17  # (b+1) + (16-b) block units per core
G0 = 9  # slots 0..8 -> PE rows 0:64, slots 9..16 -> PE rows 64:128
NPAIR = 9
KSUB = 128  # key subtile (psum partition dim)
NKT = BLK // KSUB  # 4
VW = NKT * 65  # 260
QKW = 2 * BLK  # 1024 cols of q|k per pair
INW = QKW + 2 * VW  # 1544 cols per fused input row

# diagonal pairs (0,1) have longer exp->mask->AV chains: keep them off the
# cold start and off the drain tail
PAIR_ORDER = [2, 3, 0, 1, 4, 5, 6, 7, 8]

_BF16 = ml_dtypes.bfloat16
_CACHE = {}

# cubic-in-t fit of exp(t/128) (chebyshev nodes, |t|<=56); the DVE op
# squares it to get exp(t/64). Max rel err ~5.5e-4 for |t|<=56.
_EXPC = (8.02364796e-08, 3.10070749e-05, 7.81220049e-03, 9.99807965e-01)


def _register_exp_dve_op():
    """Register a custom DVE op: out = (((x*c3 + c2)*x + c1)*x + c0)^2.

    One DVE instruction evaluates exp(x/64) to ~5e-4 rel err, letting the
    Vector engine share softmax-exp work with the Scalar engine (the
    per-element-throughput bottleneck of this kernel).
    """
    import numpy as np
    from concourse import dve_ops
    from concourse.dve_spec import (
        Spec, Src0, C0, C1, C2, C3, _spill_c3_to_src1, lower, _has_src1, sq,
    )
    from concourse.dve_uop import DveOpSpec

    name = "EXP_SQ_ANT"
    if name in dve_ops._SUB_OPCODE_FOR_NAME:
        return next(o for o in dve_ops.OPS if o.name == name)

    body = _spill_c3_to_src1(
        sq(((Src0 * C0 + C1) * Src0 + C2) * Src0 + C3))

    def ref(in0, in1, s0, s1, imm2):
        x = in0.astype(np.float32)
        p = ((x * s0 + s1) * x + imm2) * x + in1
        return (p * p).astype(np.float32)

    spec = Spec(body=body, reference=ref)
    row = dve_ops._CUSTOM_DVE_ROW_BASE + len(dve_ops.OPS)
    assert row < 0x20
    shas = {}
    for ver in ("v3",):
        s = DveOpSpec(name=name, opcode=row, uops=lower(spec, ver=ver),
                      rd1_en=_has_src1(spec))
        shas[ver] = s.sha(ver)
    op = dve_ops.DveOp(name, spec, subdim=False, uops_sha=shas)
    dve_ops.OPS.append(op)
    dve_ops._SUB_OPCODE_FOR_NAME[name] = row
    dve_ops.CUSTOM_DVE_SPECS[name] = spec
    return op


def _core_slots(b):
    """Slot table for core b: list of (rowblock, keyblock, is_diag)."""
    A, B = b, 15 - b
    slots = [(A, A, True), (B, B, True)]
    slots += [(A, c, False) for c in range(A)]
    slots += [(B, c, False) for c in range(B)]
    assert len(slots) == NSLOT
    return slots


def _build_graph():
    import concourse.mybir as mybir
    import concourse.tile as tile
    from concourse import bacc

    f32 = mybir.dt.float32
    bf16 = mybir.dt.bfloat16

    exp_op = _register_exp_dve_op()
    d3, d2, d1, d0 = _EXPC

    nc = bacc.Bacc("TRN2", target_bir_lowering=False)
    # fused per-pair input rows, laid out in PAIR_ORDER position order:
    # cols [0:512) q^T  [512:1024) k^T  [1024:1544) v_aug strips
    inp = nc.declare_dram_parameter("inp", [NPAIR, 128, INW], bf16,
                                    isOutput=False)
    op = nc.declare_dram_parameter("op", [NPAIR, 65, 2 * BLK], bf16,
                                   isOutput=True)

    with tile.TileContext(nc) as tc:
        with (
            tc.tile_pool(name="data", bufs=1) as data,
            tc.tile_pool(name="stp", bufs=3, space="PSUM") as stp,
            tc.tile_pool(name="avp", bufs=2, space="PSUM") as avp,
            tc.tile_pool(name="sxp", bufs=10) as sxp,
            tc.tile_pool(name="outp", bufs=2) as outp,
        ):
            # fused input tiles; the first pair's qk leading columns
            # (q full + k kt0 strip) get their own DMA so the first
            # matmul's dependency is a single small transfer
            incol = {}
            for pos, i in enumerate(PAIR_ORDER):
                t = data.tile([128, INW], bf16, tag=f"in{i}", name=f"in{i}")
                if pos == 0:
                    # the very first QK matmul (slot a, PE rows 0:64) only
                    # needs partitions 0:64 of the q + k-kt0 columns: give
                    # that 80KB strip its own DMA so compute starts ASAP
                    nc.sync.dma_start(out=t[0:64, 0:BLK + KSUB],
                                      in_=inp[pos][0:64, 0:BLK + KSUB])
                    nc.sync.dma_start(out=t[64:128, 0:BLK + KSUB],
                                      in_=inp[pos][64:128, 0:BLK + KSUB])
                    nc.sync.dma_start(out=t[:, BLK + KSUB:QKW],
                                      in_=inp[pos][:, BLK + KSUB:QKW])
                    nc.sync.dma_start(out=t[:, QKW:INW],
                                      in_=inp[pos][:, QKW:INW])
                else:
                    nc.sync.dma_start(out=t, in_=inp[pos])
                incol[i] = t
            d0col = data.tile([128, 1], f32, tag="d0col", name="d0col")
            nc.vector.memset(d0col, d0)

            def emit_av_out(pos, i, slots, sxs):
                """AV matmuls + per-slot copies + output DMA for a pair."""
                ot = outp.tile([65, 2 * BLK], bf16, tag="ot", name=f"ot{i}")
                for s in slots:
                    av = avp.tile([65, BLK], f32, tag="av",
                                  name=f"av{i}s{s}")
                    off = 0 if s == i else BLK
                    voff = QKW + (0 if s == i else VW)
                    for kt in range(NKT):
                        roff = KSUB * kt if s < 2 else 0
                        nc.tensor.matmul(
                            av[:, roff:BLK],
                            incol[i][:, voff + kt * 65:voff + kt * 65 + 65],
                            sxs[s][kt][:, roff:BLK],
                            start=(kt == 0),
                            stop=(kt == NKT - 1),
                        )
                    if s == i:
                        # slot a -> Vector, slot b -> Scalar: balances the
                        # PSUM->SBUF copy work across the two engines
                        nc.vector.tensor_copy(ot[:, off:off + BLK], av)
                    else:
                        nc.scalar.copy(ot[:, off:off + BLK], av)
                w = 2 * BLK if len(slots) == 2 else BLK
                nc.sync.dma_start(out=op[pos][:, 0:w], in_=ot[:, 0:w])

            # software-pipelined emission: pair p's AV stage is emitted
            # after pair p+1's QK+exp stage, so in the PE queue the two
            # row-half QK matmuls of each subtile stay adjacent (they run
            # concurrently in the array) and AVs fill dependency stalls
            prev = None
            for pos, i in enumerate(PAIR_ORDER):
                slots = [i] + ([9 + i] if 9 + i < NSLOT else [])
                sxs = {s: [] for s in slots}
                for kt in range(NKT):
                    # one PSUM tile holds this subtile's scores for BOTH
                    # slots of the pair (one bank each) -> single pool
                    # allocation, so the two row-half matmuls stay adjacent
                    # in the PE queue and run concurrently in the array
                    st = stp.tile([128, 2 * BLK], f32, tag="st",
                                  name=f"st{i}k{kt}")
                    # diagonal slots (0/1, always in the g0 half): rows
                    # < 128*kt of subtile kt are fully masked - skip them
                    # in QK, exp and AV
                    roff = KSUB * kt if i < 2 else 0
                    for s in slots:
                        p0 = 0 if s < G0 else 64
                        off = 0 if s == i else BLK
                        ro = roff if s == i else 0
                        nc.tensor.matmul(
                            st[:, off + ro:off + BLK],
                            incol[i][p0:p0 + 64,
                                     BLK + kt * KSUB:BLK + (kt + 1) * KSUB],
                            incol[i][p0:p0 + 64, ro:BLK],
                            start=True,
                            stop=True,
                            tile_position=(p0, 0),
                        )
                    sx = sxp.tile([128, 2 * BLK], bf16, tag="sx",
                                  name=f"sx{i}k{kt}")
                    if len(slots) == 2 and roff == 0:
                        spans = [(0, 2 * BLK)]
                    elif len(slots) == 2:
                        spans = [(roff, BLK), (BLK, 2 * BLK)]
                    else:
                        spans = [(roff, BLK)]
                    # kt0/kt2 -> Vector (cubic^2 custom op), kt1/kt3 ->
                    # Scalar (exact): two exps in flight per pair and an
                    # even per-pair split of the elementwise work
                    for lo, hi in spans:
                        if kt % 2 == 0:
                            nc.vector._custom_dve(
                                exp_op, out=sx[:, lo:hi],
                                in0=st[:, lo:hi], in1=d0col,
                                s0=d3, s1=d2, imm2=d1,
                            )
                        else:
                            nc.scalar.activation(
                                sx[:, lo:hi], st[:, lo:hi],
                                mybir.ActivationFunctionType.Exp,
                                scale=1.0 / DK,
                            )
                    if i < 2:  # diagonal slot: zero keys > row
                        nc.gpsimd.affine_select(
                            out=sx[:, roff:BLK],
                            in_=sx[:, roff:BLK],
                            pattern=[[1, BLK - roff]],
                            compare_op=mybir.AluOpType.is_ge,
                            fill=0.0,
                            base=0,
                            channel_multiplier=-1,
                        )
                    for s in slots:
                        off = 0 if s == i else BLK
                        sxs[s].append(sx[:, off:off + BLK])
                if prev is not None:
                    emit_av_out(*prev)
                prev = (pos, i, slots, sxs)
            emit_av_out(*prev)

    nc.finalize()
    return nc


def _pack_core(q_bf, k_bf, v_bf, b):
    """Build the packed fused operand array for core b."""
    inp = np.zeros((NPAIR, 128, INW), dtype=_BF16)
    slots = _core_slots(b)
    pos_of = {i: pos for pos, i in enumerate(PAIR_ORDER)}
    for s, (rb, cb, _diag) in enumerate(slots):
        i = s if s < G0 else s - G0
        pos = pos_of[i]
        p0 = 0 if s < G0 else 64
        voff = QKW + (0 if s < G0 else VW)
        inp[pos, p0:p0 + 64, 0:BLK] = q_bf[rb * BLK:(rb + 1) * BLK].T
        inp[pos, p0:p0 + 64, BLK:QKW] = k_bf[cb * BLK:(cb + 1) * BLK].T
        for kt in range(NKT):
            c0 = voff + kt * 65
            inp[pos, :, c0:c0 + 64] = (
                v_bf[cb * BLK + kt * KSUB: cb * BLK + (kt + 1) * KSUB])
            inp[pos, :, c0 + 64] = np.asarray(1.0, dtype=_BF16)
    return {"inp": inp}


def _slot_partial(op_arr, s):
    """Extract slot s's [65, 512] partial from the per-pair output array."""
    i = s if s < G0 else s - G0
    pos = PAIR_ORDER.index(i)
    off = 0 if s < G0 else BLK
    return op_arr[pos, :, off:off + BLK]


def _combine(partials):
    """partials: list of 8 arrays [9, 65, 1024] -> full [8192, 64] f32."""
    out = np.empty((S, DK), dtype=np.float32)
    for b in range(N_CORES):
        slots = _core_slots(b)
        for rb in (b, 15 - b):
            idx = [s for s, (r, _c, _d) in enumerate(slots) if r == rb]
            tot = np.zeros((65, BLK), dtype=np.float32)
            for s in idx:
                tot += _slot_partial(partials[b], s).astype(np.float32)
            out[rb * BLK:(rb + 1) * BLK] = (tot[:DK] / tot[DK]).T
    return out


def kernel(q, k, v):
    from concourse.bass_utils import run_bass_kernel_spmd

    q = np.asarray(q, dtype=np.float32)
    k = np.asarray(k, dtype=np.float32)
    v = np.asarray(v, dtype=np.float32)

    if "nc" not in _CACHE:
        _CACHE["nc"] = _build_graph()
    nc = _CACHE["nc"]

    q_bf = q.astype(_BF16)
    k_bf = k.astype(_BF16)
    v_bf = v.astype(_BF16)
    in_maps = [_pack_core(q_bf, k_bf, v_bf, b) for b in range(N_CORES)]

    res = run_bass_kernel_spmd(nc, in_maps, core_ids=list(range(N_CORES)))
    partials = [np.asarray(res.results[b]["op"]) for b in range(N_CORES)]
    return _combine(partials)
